# revision 2
# baseline (speedup 1.0000x reference)
"""Trainium2 kernel for BIMBlockND (nn_APUNet_33079838114069).

The reference computes, per batch n:
    xf = im2col(x)                      # (C*P, K*K) with P=256 patches
    out = g_weight @ xf  (1x1 conv)     # (8192, 64)
    scatter-back + residual add

Folding all batches into one GEMM column dim and the residual into the
weights (W' = g + I), the whole problem is:

    Out[8192, 1024] = (g + I) @ Xf,   Xf[i, n*64+s] = im2col(x)

Sharding: tensor-parallel over the 8192 output rows across 8 cores
(1024 rows each).  Every core gets the full Xf (host-side im2col) and
its W'^T shard; no collectives needed.  Compute in bf16 (1 cycle/row on
the PE vs 4 for f32; rel-err ~2e-3, well under the 2e-2 gate).
"""

import numpy as np
import ml_dtypes

B, C, H, W = 16, 32, 128, 128
K = 8
HP = WP = 16
P = HP * WP          # 256 patches
CI = C * P           # 8192 channels in GEMM space
NCORES = 8
MS = CI // NCORES    # 1024 output rows per core
NCOL = B * K * K     # 1024 GEMM columns (batch x intra-patch position)
PTILE = 128          # partition tile
NTILE = 512          # psum bank free size (f32)
KT = CI // PTILE     # 64 k-tiles
MT = MS // PTILE     # 8 m-tiles
NB = NCOL // NTILE   # 2 n-blocks

_NC = None


def _build_nc():
    from concourse import bacc, tile
    import concourse.mybir as mybir

    nc = bacc.Bacc("TRN2", target_bir_lowering=False, debug=False,
                   num_devices=NCORES)
    wt = nc.declare_dram_parameter("wt", [CI, MS], mybir.dt.bfloat16,
                                   isOutput=False)
    xf = nc.declare_dram_parameter("xf", [CI, NCOL], mybir.dt.bfloat16,
                                   isOutput=False)
    out = nc.declare_dram_parameter("out", [MS, NCOL], mybir.dt.float32,
                                    isOutput=True)

    with tile.TileContext(nc) as tc:
        with (
            tc.tile_pool(name="wtp", bufs=1) as wtp,
            tc.tile_pool(name="xfp", bufs=6) as xfp,
            tc.tile_pool(name="outp", bufs=4) as outp,
            tc.tile_pool(name="pp", bufs=1, space="PSUM") as pp,
        ):
            wt_tiles = [None] * KT
            for nb in range(NB):
                psums = []
                for m in range(MT):
                    pt = pp.tile([PTILE, NTILE], mybir.dt.float32,
                                 name=f"ps_{nb}_{m}", tag=f"ps{m}", bufs=1)
                    psums.append(pt)
                for kt in range(KT):
                    if nb == 0:
                        wt_tiles[kt] = wtp.tile(
                            [PTILE, MS], mybir.dt.bfloat16,
                            name=f"wt_{kt}", tag=f"wt{kt}", bufs=1)
                        nc.sync.dma_start(
                            wt_tiles[kt][:],
                            wt[kt * PTILE:(kt + 1) * PTILE, :])
                    xt = xfp.tile([PTILE, NTILE], mybir.dt.bfloat16,
                                  name=f"xf_{nb}_{kt}", tag="xf", bufs=6)
                    nc.sync.dma_start(
                        xt[:],
                        xf[kt * PTILE:(kt + 1) * PTILE,
                           nb * NTILE:(nb + 1) * NTILE])
                    for m in range(MT):
                        nc.tensor.matmul(
                            psums[m][:],
                            wt_tiles[kt][:, m * PTILE:(m + 1) * PTILE],
                            xt[:],
                            start=(kt == 0),
                            stop=(kt == KT - 1),
                        )
                for m in range(MT):
                    ot = outp.tile([PTILE, NTILE], mybir.dt.float32,
                                   name=f"o_{nb}_{m}", tag="o", bufs=4)
                    nc.vector.tensor_copy(ot[:], psums[m][:])
                    nc.sync.dma_start(
                        out[m * PTILE:(m + 1) * PTILE,
                            nb * NTILE:(nb + 1) * NTILE],
                        ot[:])
    nc.finalize()
    return nc


def _get_nc():
    global _NC
    if _NC is None:
        _NC = _build_nc()
    return _NC


def _make_in_maps(x, g_weight):
    x = np.asarray(x, dtype=np.float32)
    g = np.asarray(g_weight, dtype=np.float32)
    # Xf[(c,ph,pw), (n,kr,kc)] = x[n, c, ph*8+kr, pw*8+kc]
    xp = x.reshape(B, C, HP, K, WP, K).transpose(1, 2, 4, 0, 3, 5)
    Xf = np.ascontiguousarray(xp.reshape(CI, NCOL))
    # W'^T with the residual folded in: W'[o,i] = g[o,i] + (o==i)
    WT = g.T.copy()
    idx = np.arange(CI)
    WT[idx, idx] += 1.0
    WTb = WT.astype(ml_dtypes.bfloat16)
    Xfb = np.ascontiguousarray(Xf.astype(ml_dtypes.bfloat16))
    return [
        {"wt": np.ascontiguousarray(WTb[:, r * MS:(r + 1) * MS]), "xf": Xfb}
        for r in range(NCORES)
    ]


def _assemble(results):
    Out = np.concatenate([results[r]["out"] for r in range(NCORES)], axis=0)
    o6 = Out.reshape(C, HP, WP, B, K, K).transpose(3, 0, 1, 4, 2, 5)
    return np.ascontiguousarray(o6.reshape(B, C, H, W)).astype(np.float32)


def kernel(x, g_weight):
    from concourse.bass_utils import run_bass_kernel_spmd
    nc = _get_nc()
    in_maps = _make_in_maps(x, g_weight)
    res = run_bass_kernel_spmd(nc, in_maps, core_ids=list(range(NCORES)))
    return _assemble(res.results)


def kernel_timed(x, g_weight, **kwargs):
    """Like kernel() but with neuron-profile tracing; returns (out, res)."""
    from concourse.bass_utils import run_bass_kernel_spmd
    nc = _get_nc()
    in_maps = _make_in_maps(x, g_weight)
    res = run_bass_kernel_spmd(nc, in_maps, core_ids=list(range(NCORES)),
                               trace=True, **kwargs)
    return _assemble(res.results), res
